# revision 1
# baseline (speedup 1.0000x reference)
"""Trainium2 Bass kernel for CausalSelfAttention with KV cache (decode, T=8).

Sharding: tensor-parallel over heads, 2 heads per core x 8 cores.
Each core: QKV projection for its 2 heads (fp16), attention over its KV-cache
shard stored in fp8-e3m4 (halves HBM traffic vs fp16; q/p stay fp16 via
mixed-dtype matmuls), partial row-parallel out-projection (fp16). Host sums
the 8 partials and adds b_proj.

PE pipeline is software-staggered: QK of pair i+1 is emitted between the
softmax of pair i and PV of pair i, so the tensor engine never waits on the
activation/vector engines.
"""

import sys

for _p in ("/opt/trn_rl_repo", "/root/.axon_site/_ro/trn_rl_repo"):
    if _p not in sys.path:
        sys.path.insert(0, _p)

import math

import numpy as np
import ml_dtypes

import concourse.bass as bass
import concourse.tile as tile
from concourse import bacc, mybir
from concourse.masks import make_identity

# Problem shape (hardcoded; see spec)
B, T, C = 16, 8, 2048
H, D = 16, 128
MAX_SEQ = 4096
START_POS = 4088
S = MAX_SEQ  # start_pos + T
NCH = S // 128  # 32 S-chunks of 128
BT = B * T  # 128
N_CORES = 8
HPC = H // N_CORES  # heads per core = 2
KC = C // 128  # 16 contraction chunks for projections
NPAIR = B * HPC  # 32 (b, hl) attention units per core

F16 = mybir.dt.float16
F32 = mybir.dt.float32
E3 = mybir.dt.float8e3
E3NP = ml_dtypes.float8_e3m4

_CACHE = {}


def _build_nc(repeat=1):
    nc = bacc.Bacc("TRN2", target_bir_lowering=False, debug=False)

    kt = nc.dram_tensor("kt", [B, HPC, D, S], E3, kind="ExternalInput").ap()
    vv = nc.dram_tensor("vv", [B, HPC, 128, NCH, 128], E3, kind="ExternalInput").ap()
    wqk = nc.dram_tensor("wqk", [128, KC, 512], F16, kind="ExternalInput").ap()
    bqk = nc.dram_tensor("bqk", [128, 4], F32, kind="ExternalInput").ap()
    wv = nc.dram_tensor("wv", [128, KC, 256], F16, kind="ExternalInput").ap()
    bv = nc.dram_tensor("bv", [1, 256], F16, kind="ExternalInput").ap()
    xt = nc.dram_tensor("xt", [128, KC, 128], F16, kind="ExternalInput").ap()
    wp = nc.dram_tensor("wp", [128, HPC, C], F16, kind="ExternalInput").ap()
    out = nc.dram_tensor("out", [BT, C], F16, kind="ExternalOutput").ap()

    with tile.TileContext(nc) as tc:
        for _ in range(repeat):
            _emit(tc, kt, vv, wqk, bqk, wv, bv, xt, wp, out)
    nc.finalize()
    return nc


def _emit(tc, kt, vv, wqk, bqk, wv, bv, xt, wp, out):
    from contextlib import ExitStack

    nc = tc.nc
    scale = 1.0 / math.sqrt(D)

    with ExitStack() as ctx:
        const = ctx.enter_context(tc.tile_pool(name="const", bufs=1))
        kv = ctx.enter_context(tc.tile_pool(name="kv", bufs=5))
        ptp = ctx.enter_context(tc.tile_pool(name="ptp", bufs=3))
        small = ctx.enter_context(tc.tile_pool(name="small", bufs=3))
        outsb = ctx.enter_context(tc.tile_pool(name="outsb", bufs=2))
        ps_s = ctx.enter_context(tc.tile_pool(name="ps_s", bufs=2, space="PSUM"))
        ps_ya = ctx.enter_context(tc.tile_pool(name="ps_ya", bufs=2, space="PSUM"))
        ps_yt = ctx.enter_context(tc.tile_pool(name="ps_yt", bufs=1, space="PSUM"))
        ps_pso = ctx.enter_context(tc.tile_pool(name="ps_pso", bufs=1, space="PSUM"))
        ps_sums = ctx.enter_context(tc.tile_pool(name="ps_sums", bufs=1, space="PSUM"))

        # ---- constants ----
        # wqk feeds the first PE work: split across both HWDGE queues.
        # wp is only needed in phase 3: issue from gpsimd to keep the HWDGE
        # queues clear for the cache stream.
        wqk_sb = const.tile([128, KC, 512], F16)
        nc.sync.dma_start(wqk_sb[:, 0 : KC // 2, :], wqk[:, 0 : KC // 2, :])
        nc.scalar.dma_start(wqk_sb[:, KC // 2 : KC, :], wqk[:, KC // 2 : KC, :])
        xt_sb = const.tile([128, KC, 128], F16)
        nc.sync.dma_start(xt_sb[:], xt)
        bqk_sb = const.tile([128, 4], F32)
        nc.sync.dma_start(bqk_sb[:], bqk)
        wv_sb = const.tile([128, KC, 256], F16)
        nc.scalar.dma_start(wv_sb[:], wv)
        bv_sb = const.tile([1, 256], F16)
        nc.scalar.dma_start(bv_sb[:], bv)
        wp_sb = const.tile([128, HPC, C], F16)
        nc.gpsimd.dma_start(wp_sb[:], wp)
        ones1 = const.tile([1, 128], F16)
        nc.vector.memset(ones1[:], 1.0)
        ones128 = const.tile([128, 1], F32)
        nc.vector.memset(ones128[:], 1.0)
        id8 = const.tile([8, 8], F32)
        make_identity(nc, id8[:])

        # ---- phase 1: projections ----
        # qkT[m] = (w_qk[:, m-block]).T @ x.T   -> [d, b*t], m in {q0,q1,k0,k1}
        qkT_sb = const.tile([128, 4 * 128], F16)
        for m in range(4):
            ps = ps_s.tile([128, 512], F32, tag="ps_s")
            for c in range(KC):
                nc.tensor.matmul(
                    ps[:, 0:128],
                    lhsT=wqk_sb[:, c, m * 128 : (m + 1) * 128],
                    rhs=xt_sb[:, c, :],
                    start=(c == 0),
                    stop=(c == KC - 1),
                )
            nc.scalar.activation(
                qkT_sb[:, m * 128 : (m + 1) * 128],
                ps[:, 0:128],
                func=mybir.ActivationFunctionType.Identity,
                bias=bqk_sb[:, m : m + 1],
                scale=1.0,
            )

        # v_new = x @ w_v + b_v  -> [b*t, hl*128+d], cast to e3m4 for cache write
        vproj8_sb = const.tile([128, 256], E3)
        psv = ps_s.tile([128, 512], F32, tag="ps_s")
        for c in range(KC):
            nc.tensor.matmul(
                psv[:, 0:256],
                lhsT=xt_sb[:, c, :],
                rhs=wv_sb[:, c, :],
                start=(c == 0),
                stop=False,
            )
        nc.tensor.matmul(
            psv[:, 0:256], lhsT=ones1[:], rhs=bv_sb[:], start=False, stop=True
        )
        nc.vector.tensor_copy(vproj8_sb[:], psv[:, 0:256])

        # ---- phase 2: attention over 32 (b, hl) pairs, software pipelined ----
        yall_sb = const.tile([128, HPC * 128], F16)

        def load_pair(i):
            b, hl = divmod(i, HPC)
            # Partition-split DMAs: each covers [32, full-free] so descriptors
            # are 4KB (vs 1KB for column splits), and issuance is spread over
            # four engine queues so no sequencer serializes on ~650ns/DMA.
            kt_t = kv.tile([128, S], E3, tag="kt")
            for qq, eng in enumerate((nc.sync, nc.scalar)):
                eng.dma_start(
                    kt_t[qq * 64 : (qq + 1) * 64, :], kt[b, hl, qq * 64 : (qq + 1) * 64, :]
                )
            # the 8 new k columns (start_pos..start_pos+8), f16 -> e3m4 cast
            nc.vector.tensor_copy(
                kt_t[:, START_POS : START_POS + 8],
                qkT_sb[:, (2 + hl) * 128 + b * 8 : (2 + hl) * 128 + b * 8 + 8],
            )
            v_t = kv.tile([128, NCH, 128], E3, tag="v")
            for qq, eng in enumerate((nc.scalar, nc.sync)):
                eng.dma_start(
                    v_t[qq * 64 : (qq + 1) * 64, :, :],
                    vv[b, hl, qq * 64 : (qq + 1) * 64, :, :],
                )
            # the 8 new v rows (chunk 31, partitions 120..128)
            nc.gpsimd.dma_start(
                v_t[120:128, NCH - 1, 0:128],
                vproj8_sb[b * 8 : (b + 1) * 8, hl * 128 : (hl + 1) * 128],
            )
            return kt_t, v_t

        def qk(i, kt_t):
            b, hl = divmod(i, HPC)
            qT_b = qkT_sb[:, hl * 128 + b * 8 : hl * 128 + b * 8 + 8]
            ps = ps_s.tile([128, 512], F32, tag="ps_s")
            for c in range(NCH):
                nc.tensor.matmul(
                    ps[:, c * 8 : (c + 1) * 8],
                    lhsT=kt_t[:, c * 128 : (c + 1) * 128],
                    rhs=qT_b,
                    start=(c == 0),
                    stop=(c == NCH - 1),
                )
            return ps

        st = {}  # pair index -> in-flight state

        kt0, v0 = load_pair(0)
        st[0] = dict(kt=kt0, v=v0)
        kt1, v1 = load_pair(1)
        st[1] = dict(kt=kt1, v=v1)
        st[0]["ps"] = qk(0, kt0)

        for i in range(NPAIR):
            cur = st[i]
            # exp + chunk-sums for pair i (ACT + DVE)
            pt = ptp.tile([128, 256], F16)
            nc.scalar.activation(
                pt[:], cur["ps"][:, 0:256],
                func=mybir.ActivationFunctionType.Exp, scale=scale,
            )
            csum = small.tile([128, 8], F32, tag="csum")
            nc.vector.reduce_sum(
                csum[:], pt[:].rearrange("p (c t) -> p t c", t=8),
                axis=mybir.AxisListType.X,
            )
            cur["pt"], cur["csum"] = pt, csum

            # prefetch 2 ahead (DMA + k-new fixup well before QK needs them),
            # QK for pair i+1 (PE keeps streaming)
            if i + 2 < NPAIR:
                ktn, vn = load_pair(i + 2)
                st[i + 2] = dict(kt=ktn, v=vn)
            if i + 1 < NPAIR:
                st[i + 1]["ps"] = qk(i + 1, st[i + 1]["kt"])

            # finish softmax sums for pair i-1 (PE small + DVE)
            if i - 1 >= 0:
                prev = st[i - 1]
                sums = ps_sums.tile([8, 1], F32, tag="sums")
                nc.tensor.matmul(sums[:], lhsT=prev["csum"][:], rhs=ones128[:])
                rec = small.tile([8, 1], F32, tag="rec")
                nc.vector.reciprocal(rec[:], sums[:])
                yn = small.tile([8, 128], F32, tag="yn")
                nc.vector.tensor_scalar_mul(yn[:], prev["ya"][:], rec[:])
                prev["yn"] = yn

            # PV for pair i (PE big)
            ya = ps_ya.tile([8, 128], F32)
            v_t = cur["v"]
            for c in range(NCH):
                nc.tensor.matmul(
                    ya[:],
                    lhsT=cur["pt"][:, c * 8 : (c + 1) * 8],
                    rhs=v_t[:, c, :],
                    start=(c == 0),
                    stop=(c == NCH - 1),
                )
            cur["ya"] = ya

            # transpose + park y for pair i-2 (PE small + DVE)
            if i - 2 >= 0:
                bb, hh = divmod(i - 2, HPC)
                pp = st.pop(i - 2)
                yt = ps_yt.tile([128, 8], F32)
                nc.tensor.transpose(yt[:], pp["yn"][:], id8[:])
                nc.vector.tensor_copy(
                    yall_sb[:, hh * 128 + bb * 8 : hh * 128 + bb * 8 + 8], yt[:]
                )

        # drain pairs 30, 31
        for i in (NPAIR, NPAIR + 1):
            if i - 1 in st and "yn" not in st[i - 1]:
                prev = st[i - 1]
                sums = ps_sums.tile([8, 1], F32, tag="sums")
                nc.tensor.matmul(sums[:], lhsT=prev["csum"][:], rhs=ones128[:])
                rec = small.tile([8, 1], F32, tag="rec")
                nc.vector.reciprocal(rec[:], sums[:])
                yn = small.tile([8, 128], F32, tag="yn")
                nc.vector.tensor_scalar_mul(yn[:], prev["ya"][:], rec[:])
                prev["yn"] = yn
            j = i - 2
            if j in st:
                bb, hh = divmod(j, HPC)
                pp = st.pop(j)
                yt = ps_yt.tile([128, 8], F32)
                nc.tensor.transpose(yt[:], pp["yn"][:], id8[:])
                nc.vector.tensor_copy(
                    yall_sb[:, hh * 128 + bb * 8 : hh * 128 + bb * 8 + 8], yt[:]
                )

        # ---- phase 3: partial out-projection (fp16 weights, fp32 psum) ----
        for nb in range(4):
            pso = ps_pso.tile([128, 512], F32)
            for kc in range(HPC):
                nc.tensor.matmul(
                    pso[:],
                    lhsT=yall_sb[:, kc * 128 : (kc + 1) * 128],
                    rhs=wp_sb[:, kc, nb * 512 : (nb + 1) * 512],
                    start=(kc == 0),
                    stop=(kc == HPC - 1),
                )
            osb = outsb.tile([128, 512], F16)
            nc.vector.tensor_copy(osb[:], pso[:])
            nc.sync.dma_start(out[:, nb * 512 : (nb + 1) * 512], osb[:])


def _prep_core_inputs(core, x2d, k_cache, v_cache, w_attn, b_attn, w_proj):
    hg0 = HPC * core
    f16 = np.float16

    # wqk[p, c, m*128+j]: m in {q_h0, q_h1, k_h0, k_h1}
    cols = []
    for m in range(2):  # q block then k block
        for hl in range(HPC):
            base = m * C + (hg0 + hl) * D
            cols.append(np.arange(base, base + D))
    cols = np.concatenate(cols)  # [512]
    wqk = np.ascontiguousarray(
        w_attn[:, cols].reshape(KC, 128, 512).transpose(1, 0, 2)
    ).astype(f16)
    bqk = np.ascontiguousarray(b_attn[cols].reshape(4, 128).T).astype(np.float32)

    vcols = np.arange(2 * C + hg0 * D, 2 * C + (hg0 + HPC) * D)  # [256]
    wv = np.ascontiguousarray(
        w_attn[:, vcols].reshape(KC, 128, 256).transpose(1, 0, 2)
    ).astype(f16)
    bv = b_attn[vcols].reshape(1, 256).astype(f16)

    xt = np.ascontiguousarray(x2d.T.reshape(KC, 128, 128).transpose(1, 0, 2)).astype(
        f16
    )

    wpl = w_proj[hg0 * D : (hg0 + HPC) * D, :]  # [256, C]
    wp = np.ascontiguousarray(wpl.reshape(HPC, 128, C).transpose(1, 0, 2)).astype(
        np.float16
    )

    return {
        "wqk": wqk,
        "bqk": bqk,
        "wv": wv,
        "bv": bv,
        "xt": xt,
        "wp": wp,
    }


def _prep_big_concat(k_cache, v_cache):
    """Build the concatenated kt/vv arrays (e3m4) for all 8 cores, threaded."""
    from concurrent.futures import ThreadPoolExecutor

    kt_cat = np.empty((N_CORES * B, HPC, D, S), E3NP)
    vv_cat = np.empty((N_CORES * B, HPC, 128, NCH, 128), E3NP)

    def fill(job):
        core, b = job
        hg0 = HPC * core
        i = core * B + b
        kt_cat[i] = k_cache[b, hg0 : hg0 + HPC].transpose(0, 2, 1).astype(E3NP)
        vv_cat[i] = (
            v_cache[b, hg0 : hg0 + HPC]
            .reshape(HPC, NCH, 128, D)
            .transpose(0, 2, 1, 3)
            .astype(E3NP)
        )

    jobs = [(c, b) for c in range(N_CORES) for b in range(B)]
    with ThreadPoolExecutor(max_workers=16) as ex:
        list(ex.map(fill, jobs))
    return kt_cat, vv_cat


def _get_runner():
    """Compile once per process: returns (sharded_jit, in_names, mesh_sharding)."""
    if "runner" in _CACHE:
        return _CACHE["runner"]
    import jax
    from jax.sharding import Mesh, NamedSharding, PartitionSpec
    from jax.experimental.shard_map import shard_map
    from concourse import bass2jax

    nc = _build_nc()
    bass2jax.install_neuronx_cc_hook()
    partition_name = nc.partition_id_tensor.name if nc.partition_id_tensor else None

    in_names, out_names, out_avals, zero_outs = [], [], [], []
    for alloc in nc.m.functions[0].allocations:
        if not isinstance(alloc, mybir.MemoryLocationSet):
            continue
        name = alloc.memorylocations[0].name
        if alloc.kind == "ExternalInput":
            if name != partition_name:
                in_names.append(name)
        elif alloc.kind == "ExternalOutput":
            out_names.append(name)
            shape = tuple(alloc.tensor_shape)
            dtype = mybir.dt.np(alloc.dtype)
            out_avals.append(jax.core.ShapedArray(shape, dtype))
            zero_outs.append(np.zeros(shape, dtype))
    n_params = len(in_names)
    all_in_names = list(in_names) + list(out_names)
    if partition_name is not None:
        all_in_names.append(partition_name)

    def _body(*args):
        operands = list(args)
        if partition_name is not None:
            operands.append(bass2jax.partition_id_tensor())
        outs = bass2jax._bass_exec_p.bind(
            *operands,
            out_avals=tuple(out_avals),
            in_names=tuple(all_in_names),
            out_names=tuple(out_names),
            lowering_input_output_aliases=(),
            sim_require_finite=True,
            sim_require_nnan=True,
            nc=nc,
        )
        return tuple(outs)

    devices = jax.devices()[:N_CORES]
    mesh = Mesh(np.asarray(devices), ("core",))
    in_specs = (PartitionSpec("core"),) * (n_params + len(out_names))
    out_specs = (PartitionSpec("core"),) * len(out_names)
    sharded = jax.jit(
        shard_map(_body, mesh=mesh, in_specs=in_specs, out_specs=out_specs,
                  check_rep=False),
        keep_unused=True,
    )
    sh = NamedSharding(mesh, PartitionSpec("core"))
    dev_zeros = [
        jax.device_put(np.zeros((N_CORES * z.shape[0], *z.shape[1:]), z.dtype), sh)
        for z in zero_outs
    ]
    _CACHE["runner"] = (sharded, in_names, out_names, out_avals, sh, dev_zeros)
    return _CACHE["runner"]


def kernel(
    x,
    k_cache,
    v_cache,
    w_attn,
    b_attn,
    w_proj,
    b_proj,
    start_pos,
    is_causal,
):
    x = np.asarray(x, dtype=np.float32)
    k_cache = np.asarray(k_cache, dtype=np.float32)
    v_cache = np.asarray(v_cache, dtype=np.float32)
    w_attn = np.asarray(w_attn, dtype=np.float32)
    b_attn = np.asarray(b_attn, dtype=np.float32)
    w_proj = np.asarray(w_proj, dtype=np.float32)
    b_proj = np.asarray(b_proj, dtype=np.float32)
    assert int(start_pos) == START_POS, f"kernel hardcodes start_pos={START_POS}"
    assert int(is_causal) == 0, "kernel hardcodes is_causal=0"

    sharded, in_names, out_names, out_avals, sh, dev_zeros = _get_runner()

    x2d = x.reshape(BT, C)
    kt_cat, vv_cat = _prep_big_concat(k_cache, v_cache)
    in_maps = [
        _prep_core_inputs(c, x2d, k_cache, v_cache, w_attn, b_attn, w_proj)
        for c in range(N_CORES)
    ]
    big = {"kt": kt_cat, "vv": vv_cat}
    concat_in = [
        big[nm]
        if nm in big
        else np.concatenate([in_maps[c][nm] for c in range(N_CORES)], axis=0)
        for nm in in_names
    ]
    outs = sharded(*concat_in, *dev_zeros)
    partial = (
        np.asarray(outs[0])
        .astype(np.float64)
        .reshape(N_CORES, BT, C)
        .sum(axis=0)
    )
    y = (partial + b_proj).astype(np.float32)
    return y.reshape(B, T, C)


if __name__ == "__main__":
    # quick self-run against the local reference
    sys.path.insert(0, "/root/problem")
    import reference

    inputs = {k: np.asarray(v) for k, v in reference.setup_inputs().items()}
    expected = np.asarray(reference.reference(**reference.setup_inputs()))
    actual = kernel(**inputs)
    err = np.abs(actual - expected)
    rel = err.max() / np.abs(expected).max()
    print("max abs err:", err.max(), "rel:", rel)

